# revision 46
# baseline (speedup 1.0000x reference)
"""Trainium2 Bass kernel for MQA causal attention (16 q heads, 1 shared kv head).

Sharding (hybrid, per the hint): 2-way data-parallel over batch x 4-way
tensor-parallel over query heads -> each of the 8 cores handles ONE batch
element with 4 query heads, sharing the single K/V head.  vs pure 8-way
head sharding this halves the replicated K/V projection work AND halves
per-core x/y DMA traffic.  Each core emits a bf16 partial out-projection;
the host sums the 4 partials per batch element in f32 (the all-reduce of
the hint).

Per-core structure (phases pipelined per rep):
  - x arrives dim-major (xT, bf16) so every matmul contraction dim is
    already on partitions; x tile DMAs split across the SP/ACT HWDGE queues.
  - RoPE: q_rot = q*cos + rot(q)*sin; the three multiplies read the
    projection PSUM on DVE (GPSIMD has no PSUM access), the SBUF-only add
    rides Pool.  cos/sin tables host-precomputed ([d, n] layout, q tables
    pre-scaled by 1/sqrt(d), sin pre-signed for rotate_half).
  - V is produced NATURAL ([keys, d]) directly: stat = xT key-chunk,
    moving = Wkv v-columns, 4 accumulation regions sharing one PSUM bank
    (skip_group_check).  Same PE cost as a transposed projection but no PE
    transposes and no PSUM-bank WAR against the rope DVE drains (which
    used to stall PE ~12us/rep).
  - Scores are computed transposed: simT[keys, h*q] = kT.T @ qT.  With MQA
    the k chunk is the stationary operand shared by all 4 heads, so the
    heads ride in the moving operand free dim (512-col matmuls).
  - The attention inner loop is SOFTWARE-PIPELINED: scores(c+1) issues
    before attnV(c), so the serial chain scores->exp->attnV (213+612+213ns
    with zero overlap = 1038ns/chunk) collapses to max(engine) ~650ns.
    The diagonal chunk's scores+exp go FIRST (its Pool affine_select mask
    latency hides behind the other chunks) and its attnV goes LAST.
  - softmax: exp on ScalarE, no max subtraction (measured |sim| <= 5.6);
    denominator: ex is the STATIONARY matmul operand against a 1-column
    ones vector (0 observable PE cost - LDWEIGHTS prefetch hides the
    stationary loads behind the scores/attnV streams), output
    [q-partitions, head]; a zeroing starter matmul opens the psd bank so
    the four per-head accumulation chains coexist in one zero region.
    Finalize is split: the psd->sbuf drain (DVE) issues at tile end, the
    rest (4 PE row transposes -> reciprocal -> Pool broadcast -> DVE
    normalize into bf16 attnT) at the next tile's start, so outproj
    fillers reading attnT(t-1) are unblocked early.
  - Out-projection: attnT chunks stationary, Wout slice moving; split into
    128 HALF-units per rep (2 head-chunks each, ~426ns PE - a full unit's
    852ns overshoots the 612ns/chunk exp budget) popped one per attention
    chunk (skipping each tile's first attnV, whose attnT(t-1) writer has
    only just issued) plus one at tile start.  Early tiles (t<12)
    pop only above a 40-unit FIFO reserve: the late tiles are fed solely
    by already-produced units, and without the reserve they starve and
    idle on the exp stream (-6us).  Half-unit pairs share a PSUM claim,
    so pops are re-aligned to unit parity after the attention loop (a
    foreign pswork claim between the halves of a unit would corrupt the
    ring across the rep boundary).
  - psd and the transposed den row share ONE PSUM bank (psdt is a bf16
    bitcast view of the psd tile's upper half; the tag-ring claim orders
    tile t+1's zero-matmul after tile t's reciprocal read), freeing the
    8th bank for a 3rd pswork buffer (-2us of psy/psq ring stalls).

Perf history (steady-state per-rep marginal, TimelineSim; HW-graded time
tracks sim under ~1.3x shared-host noise, best quiet rounds ~=sim x0.9):
baseline 392 -> 254.6 -> 231.5us sim (8-way head shard, hybrid reshard,
stationary-ex denominator).  This revision: attention-phase software
pipelining + direct-natural V + split finalize + half-unit fillers with
late-tile reserve + merged den bank: sim 231.5 -> 207.2us/rep (PE busy
~198us = 95% utilization).  HW check: rel err 5.560e-3; HW round medians
254.9-307.6us across runs, quiet rounds 172.9-200.6us (the shared host
swings +/-20% between measurement windows; quiet rounds track sim).
Tried and rejected: fp8e4 DoubleRow datapath (2.4-7.6% rel err, over the
2e-2 gate - e4m3's 3 mantissa bits put ~2.6% on every operand, and the
Double-FP8 path upcasts to e6m3 so e3m4 doesn't help), scheduler-driven
unit emission without the pop FIFO (needs units issued after finalize;
bulk emission then mis-paces the chunk cadence: 235-238us), psdt borrowed
from the scores/pswork rings to free a bank (ring-dependency poison:
237-247us), SBUF->SBUF transposed-AP DMA for the denominator row (APs
can't partition-transpose: reads garbage), K/V seq-sharding via
collectives (15us fixed overhead + an 8-way per-rep barrier under +/-40%
host noise eats the 20us PE saving), high_priority on exp/finalize and
emission-order permutations (the runahead list scheduler normalizes
both; only dependency/resource structure matters), slice/tile-quad
interleaving (+3us), per-chunk den matmul elision (sequencer dispatch
is not a limiter).  Remaining known PE
fat: 4-way K/V replication (~20us/rep) and ~3us of causal slack (not
recoverable at 128-wide tiles: matmul cost is moving-column-bound);
everything else is at the bf16 math floor (q/out projections 54.6us each,
scores/attnV 29us each).
"""

import os
import sys
from contextlib import ExitStack

import numpy as np

for _p in ("/opt/trn_rl_repo",):
    if os.path.isdir(_p) and _p not in sys.path:
        sys.path.insert(0, _p)

import ml_dtypes

import concourse.bass as bass
import concourse.mybir as mybir
import concourse.tile as tile
from concourse import bacc
from concourse.bass_utils import run_bass_kernel_spmd
from concourse.masks import make_identity

HEADS = 16
D = 128
SCALE = D ** -0.5
N_CORES = 8
BGROUPS = 2                  # batch splits
HGROUPS = N_CORES // BGROUPS  # head-group splits per batch element

F32 = mybir.dt.float32
BF16 = mybir.dt.bfloat16


def _rope(nc, sb_pool, ps, out_slice, cos_s, sin_s):
    """out_slice(bf16) = ps*cos_s + rot(ps)*sin_s. The three multiplies read
    PSUM so they must ride DVE (GPSIMD cannot access PSUM on hardware); the
    SBUF-only final add goes to Pool to shorten the DVE chain. sin_s arrives
    pre-signed from the host (rows 0-63 negated)."""
    L = ps.shape[-1]
    t1 = sb_pool.tile([128, L], F32, tag="ropet1")
    nc.vector.tensor_mul(t1, ps, cos_s)
    t2 = sb_pool.tile([128, L], F32, tag="ropet2")
    nc.vector.tensor_mul(t2[0:64, :], ps[64:128, :], sin_s[0:64, :])
    nc.vector.tensor_mul(t2[64:128, :], ps[0:64, :], sin_s[64:128, :])
    nc.gpsimd.tensor_add(out_slice, t1, t2)


def build_nc(N, DIM, HL, reps=1):
    """One SPMD program: HL query heads + shared kv head of ONE batch
    element, full sequence.

    reps>1 repeats the whole computation (same output) for timing-by-
    difference: NEFF(reps=K) wall minus NEFF(reps=1) wall = (K-1) * body.
    """
    DC = DIM // 128           # contraction chunks for projections
    SL = min(N, 512)          # projection n-slice length
    NS = N // SL              # n slices
    NKC = N // 128            # 128-wide key chunks
    NQT = N // 128            # 128-row query tiles
    KPS = SL // 128           # key chunks per slice

    nc = bacc.Bacc(None, target_bir_lowering=False)
    xT = nc.declare_dram_parameter("xT", [DIM, N], BF16, isOutput=False)
    wq = nc.declare_dram_parameter("wq", [DIM, HL * D], BF16, isOutput=False)
    wkv = nc.declare_dram_parameter("wkv", [DIM, 2 * D], BF16, isOutput=False)
    wout = nc.declare_dram_parameter("wout", [HL * D, DIM], BF16, isOutput=False)
    cosq = nc.declare_dram_parameter("cosq", [D, N], BF16, isOutput=False)
    sinq = nc.declare_dram_parameter("sinq", [D, N], BF16, isOutput=False)
    cosk = nc.declare_dram_parameter("cosk", [D, N], BF16, isOutput=False)
    sink = nc.declare_dram_parameter("sink", [D, N], BF16, isOutput=False)
    # bf16 partials: the host sums 4 of them in f32; quantization error
    # (~0.4% rel) is well inside the 2e-2 gate and halves the y DMA traffic.
    y = nc.declare_dram_parameter("y", [N, DIM], BF16, isOutput=True)

    with ExitStack() as ctx:
        tc = ctx.enter_context(tile.TileContext(nc))
        consts = ctx.enter_context(tc.tile_pool(name="consts", bufs=1))
        xpool = ctx.enter_context(tc.tile_pool(name="xpool", bufs=2))
        proj = ctx.enter_context(tc.tile_pool(name="proj", bufs=2))
        sb = ctx.enter_context(tc.tile_pool(name="sb", bufs=3))
        outp = ctx.enter_context(tc.tile_pool(name="outp", bufs=2))
        # 8 PSUM banks: scores 2, proj/outproj work 3, psa 2, psd(+psdt view) 1
        ps_sc = ctx.enter_context(tc.tile_pool(name="ps_sc", bufs=2, space="PSUM"))
        ps_work = ctx.enter_context(tc.tile_pool(name="ps_work", bufs=3, space="PSUM"))
        ps_att = ctx.enter_context(tc.tile_pool(name="ps_att", bufs=2, space="PSUM"))
        ps_den = ctx.enter_context(tc.tile_pool(name="ps_den", bufs=1, space="PSUM"))

        ident = consts.tile([128, 128], BF16)
        make_identity(nc, ident)
        ones_col = consts.tile([128, 1], BF16)
        nc.vector.memset(ones_col, 1.0)
        zeros_h = consts.tile([128, HL], BF16)
        nc.vector.memset(zeros_h, 0.0)

        wq_sb = consts.tile([128, DC, HL * D], BF16)
        wkv_sb = consts.tile([128, DC, 2 * D], BF16)
        nc.sync.dma_start(
            wq_sb, wq.rearrange("(c p) m -> p c m", p=128))
        nc.sync.dma_start(
            wkv_sb, wkv.rearrange("(c p) m -> p c m", p=128))
        # bulk constants go on the ACT HWDGE queue so they don't delay the
        # x-tile stream on the SP queue
        wout_sb = consts.tile([128, HL, DIM], BF16)
        nc.scalar.dma_start(wout_sb, wout.rearrange("(c p) m -> p c m", p=128))
        cq_sb = consts.tile([128, N], BF16)
        sq_sb = consts.tile([128, N], BF16)
        ck_sb = consts.tile([128, N], BF16)
        sk_sb = consts.tile([128, N], BF16)
        nc.scalar.dma_start(cq_sb, cosq[:, :])
        nc.scalar.dma_start(sq_sb, sinq[:, :])
        nc.scalar.dma_start(ck_sb, cosk[:, :])
        nc.scalar.dma_start(sk_sb, sink[:, :])

        pend = []        # out-proj half-units carried across phases/reps
        npop = [0]       # pops since last parity realignment

        def pop_filler(on_act=False, reserve=0):
            if len(pend) > reserve:
                pend.pop(0)(on_act)
                npop[0] += 1

        for bi in range(reps):
            first = bi == 0
            qrot = proj.tile([128, HL, N], BF16, tag="qrot")
            krot = proj.tile([128, N], BF16, tag="krot")
            vnat = proj.tile([128, NKC, D], BF16, tag="vnat")
            attnT = proj.tile([128, HL, N], BF16, tag="attnT")

            def _attn_qtile(t, prev_fin, qrot=qrot, krot=krot,
                            vnat=vnat, attnT=attnT):
                # early tiles leave a reserve in the FIFO so the late tiles
                # (fed only by already-produced units) don't starve
                rsv = 40 if t < 12 else 0
                qsl = qrot[:, :, t * 128:(t + 1) * 128]
                psa = ps_att.tile([128, HL, 128], F32, tag="psa")

                def scores(j):
                    pss = ps_sc.tile([128, HL, 128], F32, tag="pss")
                    nc.tensor.matmul(pss, krot[:, j * 128:(j + 1) * 128], qsl,
                                     start=True, stop=True)
                    ex = sb.tile([128, HL, 128], BF16, name="ex",
                                 tag="exd" if j == t else "exp")
                    nc.scalar.activation(ex, pss,
                                         mybir.ActivationFunctionType.Exp)
                    if j == t:
                        # diagonal chunk: keep where qc - kp >= 0
                        nc.gpsimd.affine_select(
                            out=ex, in_=ex,
                            compare_op=mybir.AluOpType.is_ge, fill=0.0,
                            base=0, pattern=[[0, HL], [1, 128]],
                            channel_multiplier=-1)
                    return ex

                def attnv(j, ex, first_a, last_a, psd, pop=True):
                    nc.tensor.matmul(psa, vnat[:, j, :], ex,
                                     start=first_a, stop=last_a)
                    for h in range(HL):
                        nc.tensor.matmul(psd[:, h:h + 1], ex[:, h, :], ones_col,
                                         start=False,
                                         stop=(last_a and h == HL - 1),
                                         skip_group_check=True)
                    if pop:
                        pop_filler(reserve=rsv)

                # pipeline: diag scores first, then c0..c_{t-1} with attnV
                # lagging one chunk; diag attnV last (its exp+mask latency
                # hides behind the whole tile).  prev tile's finalize chain
                # issues right after the first scores: its PE transposes wait
                # on the tile-end psd drain (DVE), which s(diag) covers.
                exd = scores(t)
                if prev_fin is not None:
                    prev_fin()
                # a tile-start filler keeps PE ahead of the 2-bank scores
                # ring warmup (the 3rd pss claim waits on exp(1st) done)
                pop_filler(reserve=rsv)
                # one bank holds BOTH den views: the accumulator psd in f32
                # cols 0:HL and the transposed row psdt in bytes 1024+ (a
                # bf16 bitcast view).  The ring claim orders tile t+1's
                # zero-matmul after tile t's reciprocal read, so the regions
                # never overlap live data.  This frees a whole PSUM bank,
                # giving pswork a 3rd buffer.
                psdf = ps_den.tile([128, 512], F32, tag="psd")
                psd = psdf[:, 0:HL]
                # zeroing matmul opens psd's zero region: the four per-head
                # accumulation chains share the bank without tripping each
                # other's pending-zero marks
                nc.tensor.matmul(psd, ident, zeros_h, start=True,
                                 stop=False, skip_group_check=True)
                prev = None
                na = 0
                for j in range(t):
                    ex = scores(j)
                    if prev is not None:
                        attnv(prev[0], prev[1], first_a=(prev[0] == 0),
                              last_a=False, psd=psd, pop=(na > 0))
                        na += 1
                    prev = (j, ex)
                if prev is not None:
                    attnv(prev[0], prev[1], first_a=(prev[0] == 0),
                          last_a=False, psd=psd, pop=(na > 0))
                    na += 1
                attnv(t, exd, first_a=(t == 0), last_a=True, psd=psd)
                # early half of finalize: drain psd while its bank cools
                sd = sb.tile([128, HL], BF16, tag="sd")
                nc.vector.tensor_copy(sd, psd)

                def finalize(t=t, psa=psa, sd=sd, psdf=psdf):
                    # flip den to row orientation: four [128,1]->[1,128] PE
                    # transposes land side by side in one psum row (lazy
                    # has_written zeroing keeps earlier columns intact)
                    psdt = psdf[0:1, 256:512].bitcast(BF16).rearrange(
                        "p (h q) -> p h q", h=HL)
                    for h in range(HL):
                        nc.tensor.transpose(psdt[0:1, h, :], sd[:, h:h + 1],
                                            ident)
                    den = sb.tile([1, HL, 128], F32, tag="den")
                    nc.vector.reciprocal(den, psdt)
                    bc = sb.tile([128, HL, 128], F32, tag="bc")
                    nc.gpsimd.partition_broadcast(bc, den)
                    nc.vector.tensor_mul(attnT[:, :, t * 128:(t + 1) * 128],
                                         psa, bc)
                return finalize

            def _outproj_units(t, attnT=attnT):
                # one 128-row m-chunk of y, as 8 half-units (2 head-chunks
                # each).  The two halves of a unit share one pswork claim, so
                # they must pop with no foreign pswork claim in between (see
                # parity realignment after the attention loop).
                units = []
                state = {}

                def half(nso, hi, on_act, m=t, state=state):
                    if "ysb" not in state:
                        state["ysb"] = outp.tile([128, DIM], BF16,
                                                 name="ysb", tag="ysb")
                    ysb = state["ysb"]
                    if hi == 0:
                        state[nso] = ps_work.tile([128, 512], F32,
                                                  name="psy", tag="pswork")
                    psy = state[nso] if hi == 0 else state.pop(nso)
                    for hc in (0, 1) if hi == 0 else (2, 3):
                        nc.tensor.matmul(
                            psy, attnT[:, hc, m * 128:(m + 1) * 128],
                            wout_sb[:, hc, nso * 512:(nso + 1) * 512],
                            start=(hc == 0), stop=(hc == HL - 1))
                    if hi == 1:
                        # psum drains can't ride Pool (no PSUM access on HW);
                        # all pops happen in the attention phase where ACT is
                        # exp-saturated, so drains ride DVE
                        nc.vector.tensor_copy(
                            ysb[:, nso * 512:(nso + 1) * 512], psy)
                        if nso == DIM // 512 - 1:
                            deng = nc.sync if m % 2 == 0 else nc.scalar
                            deng.dma_start(y[m * 128:(m + 1) * 128, :], ysb)

                for nso in range(DIM // 512):
                    for hi in (0, 1):
                        units.append(
                            lambda on_act=False, nso=nso, hi=hi: half(nso, hi, on_act))
                return units

            # ---- projections + rope, one n-slice at a time ----
            def _slice(ns):
                sl = slice(ns * SL, (ns + 1) * SL)
                xt = xpool.tile([128, DC, SL], BF16, tag="xt")
                h_dc = DC // 2
                xt_src = xT.rearrange("(c p) n -> p c n", p=128)[:, :, sl]
                nc.sync.dma_start(xt[:, :h_dc, :], xt_src[:, :h_dc, :])
                eng2 = nc.sync if (first and ns == 0) else nc.scalar
                eng2.dma_start(xt[:, h_dc:, :], xt_src[:, h_dc:, :])
                # v first, produced natural: stat = xT key-chunk, moving =
                # Wkv v-columns; 4 accumulation regions share one bank.  The
                # ACT drains land during the q matmuls.
                psvn = ps_work.tile([128, KPS, D], F32, tag="pswork")
                for kc in range(KPS):
                    for dc in range(DC):
                        nc.tensor.matmul(
                            psvn[:, kc, :], xt[:, dc, kc * 128:(kc + 1) * 128],
                            wkv_sb[:, dc, D:2 * D],
                            start=(dc == 0), stop=(dc == DC - 1),
                            skip_group_check=True)
                    nc.scalar.copy(vnat[:, ns * KPS + kc, :], psvn[:, kc, :])

                def qproj(hq):
                    psq = ps_work.tile([128, SL], F32, tag="pswork")
                    for dc in range(DC):
                        nc.tensor.matmul(
                            psq, wq_sb[:, dc, hq * D:(hq + 1) * D], xt[:, dc, :],
                            start=(dc == 0), stop=(dc == DC - 1))
                    _rope(nc, sb, psq, qrot[:, hq, sl],
                          cq_sb[:, sl], sq_sb[:, sl])

                for hq in range(HL):
                    qproj(hq)
                psk = ps_work.tile([128, SL], F32, tag="pswork")
                for dc in range(DC):
                    nc.tensor.matmul(
                        psk, wkv_sb[:, dc, 0:D], xt[:, dc, :],
                        start=(dc == 0), stop=(dc == DC - 1))
                _rope(nc, sb, psk, krot[:, sl],
                      ck_sb[:, sl], sk_sb[:, sl])

            # ---- attention tiles, software-pipelined ----
            # units(t) enter the FIFO after tile t and pop from tile t+1 on,
            # so finalize(t) has always issued before a unit reads attnT(t)
            fin = None
            npop[0] = 0
            for ns in range(NS):
                _slice(ns)
            for t in range(NQT):
                fin = _attn_qtile(t, fin)
                pend.extend(_outproj_units(t))
            # realign pops to unit parity so the projection-phase pop pairs
            # never split a unit across foreign pswork claims
            if npop[0] % 2 == 1:
                pop_filler()
            fin()
            if bi == reps - 1:
                while pend:
                    pop_filler()

    nc.finalize()
    return nc


def make_host_inputs(x, Wq, Wkv, Wout, HL):
    """Shard + precompute per-core input maps (host side)."""
    B, N, DIM = x.shape
    bf = ml_dtypes.bfloat16
    inv = 1.0 / (10000.0 ** (np.arange(0, D, 2, dtype=np.float64) / D))
    fr = np.arange(N, dtype=np.float64)[:, None] * inv[None, :]
    pos = np.concatenate([fr, fr], axis=-1)              # [N, D]
    cos_t = np.cos(pos).T.astype(np.float32)             # [D, N]
    sin_t = np.sin(pos).T.astype(np.float32)
    sign = np.ones((D, 1), np.float32)
    sign[:D // 2] = -1.0
    sin_r = sin_t * sign            # fold rotate_half's sign into the table
    shared = dict(
        wkv=Wkv.astype(bf),
        cosq=np.ascontiguousarray(cos_t * SCALE).astype(bf),
        sinq=np.ascontiguousarray(sin_r * SCALE).astype(bf),
        cosk=cos_t.astype(bf), sink=sin_r.astype(bf))
    xTs = [np.ascontiguousarray(x[b].T).astype(bf) for b in range(B)]
    in_maps = []
    for c in range(N_CORES):
        b, g = (c // HGROUPS) % B, c % HGROUPS
        lo, hi = g * HL * D, (g + 1) * HL * D
        in_maps.append(dict(
            shared, xT=xTs[b],
            wq=np.ascontiguousarray(Wq[:, lo:hi]).astype(bf),
            wout=np.ascontiguousarray(Wout[lo:hi, :]).astype(bf)))
    return in_maps


def kernel(x, Wq, Wkv, Wout):
    B, N, DIM = x.shape
    HL = HEADS // HGROUPS
    nc = build_nc(N, DIM, HL)
    in_maps = make_host_inputs(x, Wq, Wkv, Wout, HL)
    res = run_bass_kernel_spmd(nc, in_maps, core_ids=list(range(N_CORES)))
    y = np.zeros((B, N, DIM), np.float32)
    for c, r in enumerate(res.results):
        y[c // HGROUPS] += r["y"].astype(np.float32)
    return y


# revision 54
# speedup vs baseline: 1.0718x; 1.0718x over previous
"""Trainium2 Bass kernel for MQA causal attention (16 q heads, 1 shared kv head).

Sharding (hybrid, per the hint): 2-way data-parallel over batch x 4-way
tensor-parallel over query heads -> each of the 8 cores handles ONE batch
element with 4 query heads, sharing the single K/V head.  vs pure 8-way
head sharding this halves the replicated K/V projection work AND halves
per-core x/y DMA traffic.  Each core emits a bf16 partial out-projection;
the host sums the 4 partials per batch element in f32 (the all-reduce of
the hint).

Per-core structure (phases pipelined per rep):
  - x arrives dim-major (xT, bf16) so every matmul contraction dim is
    already on partitions; x tile DMAs split across the SP/ACT HWDGE queues.
  - RoPE: q_rot = q*cos + rot(q)*sin; the three multiplies read the
    projection PSUM on DVE (GPSIMD has no PSUM access), the SBUF-only add
    rides Pool.  cos/sin tables host-precomputed ([d, n] layout, q tables
    pre-scaled by 1/sqrt(d), sin pre-signed for rotate_half).
  - V is produced NATURAL ([keys, d]) directly: stat = xT key-chunk,
    moving = Wkv v-columns, 4 accumulation regions sharing one PSUM bank
    (skip_group_check).  Same PE cost as a transposed projection but no PE
    transposes and no PSUM-bank WAR against the rope DVE drains (which
    used to stall PE ~12us/rep).
  - Scores are computed transposed: simT[keys, h*q] = kT.T @ qT.  With MQA
    the k chunk is the stationary operand shared by all 4 heads, so the
    heads ride in the moving operand free dim (512-col matmuls).
  - The attention inner loop is SOFTWARE-PIPELINED: scores(c+1) issues
    before attnV(c), so the serial chain scores->exp->attnV (213+612+213ns
    with zero overlap = 1038ns/chunk) collapses to max(engine) ~650ns.
    The diagonal chunk's scores+exp go FIRST (its Pool affine_select mask
    latency hides behind the other chunks) and its attnV goes LAST.
  - softmax: exp on ScalarE, no max subtraction (measured |sim| <= 5.6);
    denominator: ex is the STATIONARY matmul operand against a 1-column
    ones vector (0 observable PE cost - LDWEIGHTS prefetch hides the
    stationary loads behind the scores/attnV streams), output
    [q-partitions, head]; a zeroing starter matmul opens the psd bank so
    the four per-head accumulation chains coexist in one zero region.
    Finalize is split: the psd->sbuf drain (DVE) issues at tile end, the
    rest (4 PE row transposes -> reciprocal -> Pool broadcast -> DVE
    normalize into bf16 attnT) at the next tile's start, so outproj
    fillers reading attnT(t-1) are unblocked early.
  - Out-projection: attnT chunks stationary, Wout slice moving; split into
    128 HALF-units per rep (2 head-chunks each, ~426ns PE - a full unit's
    852ns overshoots the 612ns/chunk exp budget) popped one per attention
    chunk (skipping each tile's first attnV, whose attnT(t-1) writer has
    only just issued) plus one at tile start.  Early tiles (t<12)
    pop only above a 40-unit FIFO reserve: the late tiles are fed solely
    by already-produced units, and without the reserve they starve and
    idle on the exp stream (-6us).  Half-unit pairs share a PSUM claim,
    so pops are re-aligned to unit parity after the attention loop (a
    foreign pswork claim between the halves of a unit would corrupt the
    ring across the rep boundary).
  - psd and the transposed den row share ONE PSUM bank (psdt is a bf16
    bitcast view of the psd tile's upper half; the tag-ring claim orders
    tile t+1's zero-matmul after tile t's reciprocal read), freeing the
    8th bank for a 3rd pswork buffer (-2us of psy/psq ring stalls).

Perf history (steady-state per-rep marginal, TimelineSim; HW-graded time
tracks sim under ~1.3x shared-host noise, best quiet rounds ~=sim x0.9):
baseline 392 -> 254.6 -> 231.5us sim (8-way head shard, hybrid reshard,
stationary-ex denominator).  This revision: attention-phase software
pipelining + direct-natural V + split finalize + half-unit fillers with
late-tile reserve + merged den bank: sim 231.5 -> 207.2us/rep (PE busy
~198us = 95% utilization).  HW check: rel err 5.560e-3; HW round medians
254.9-307.6us across runs, quiet rounds 172.9-200.6us (the shared host
swings +/-20% between measurement windows; quiet rounds track sim).
Tried and rejected: fp8e4 DoubleRow datapath (2.4-7.6% rel err, over the
2e-2 gate - e4m3's 3 mantissa bits put ~2.6% on every operand, and the
Double-FP8 path upcasts to e6m3 so e3m4 doesn't help), scheduler-driven
unit emission without the pop FIFO (needs units issued after finalize;
bulk emission then mis-paces the chunk cadence: 235-238us), psdt borrowed
from the scores/pswork rings to free a bank (ring-dependency poison:
237-247us), SBUF->SBUF transposed-AP DMA for the denominator row (APs
can't partition-transpose: reads garbage), K/V seq-sharding via
collectives (15us fixed overhead + an 8-way per-rep barrier under +/-40%
host noise eats the 20us PE saving), high_priority on exp/finalize and
emission-order permutations (the runahead list scheduler normalizes
both; only dependency/resource structure matters), slice/tile-quad
interleaving (+3us), per-chunk den matmul elision (sequencer dispatch
is not a limiter).  Remaining known PE
fat: 4-way K/V replication (~20us/rep) and ~3us of causal slack (not
recoverable at 128-wide tiles: matmul cost is moving-column-bound);
everything else is at the bf16 math floor (q/out projections 54.6us each,
scores/attnV 29us each).
"""

import os
import sys
from contextlib import ExitStack

import numpy as np

for _p in ("/opt/trn_rl_repo",):
    if os.path.isdir(_p) and _p not in sys.path:
        sys.path.insert(0, _p)

import ml_dtypes

import concourse.bass as bass
import concourse.mybir as mybir
import concourse.tile as tile
from concourse import bacc
from concourse.bass_utils import run_bass_kernel_spmd
from concourse.masks import make_identity

HEADS = 16
D = 128
SCALE = D ** -0.5
N_CORES = 8
BGROUPS = 2                  # batch splits
HGROUPS = N_CORES // BGROUPS  # head-group splits per batch element

F32 = mybir.dt.float32
BF16 = mybir.dt.bfloat16


def _rope(nc, sb_pool, ps, out_slice, cos_s, sin_s):
    """out_slice(bf16) = ps*cos_s + rot(ps)*sin_s. The three multiplies read
    PSUM so they must ride DVE (GPSIMD cannot access PSUM on hardware); the
    SBUF-only final add goes to Pool to shorten the DVE chain. sin_s arrives
    pre-signed from the host (rows 0-63 negated)."""
    L = ps.shape[-1]
    t1 = sb_pool.tile([128, L], F32, tag="ropet1")
    nc.vector.tensor_mul(t1, ps, cos_s)
    t2 = sb_pool.tile([128, L], F32, tag="ropet2")
    nc.vector.tensor_mul(t2[0:64, :], ps[64:128, :], sin_s[0:64, :])
    nc.vector.tensor_mul(t2[64:128, :], ps[0:64, :], sin_s[64:128, :])
    nc.gpsimd.tensor_add(out_slice, t1, t2)


def build_nc(N, DIM, HL, reps=1):
    """One SPMD program: HL query heads + shared kv head of ONE batch
    element, full sequence.

    reps>1 repeats the whole computation (same output) for timing-by-
    difference: NEFF(reps=K) wall minus NEFF(reps=1) wall = (K-1) * body.
    """
    DC = DIM // 128           # contraction chunks for projections
    SL = min(N, 512)          # projection n-slice length
    NS = N // SL              # n slices
    NKC = N // 128            # 128-wide key chunks
    NQT = N // 128            # 128-row query tiles
    KPS = SL // 128           # key chunks per slice

    nc = bacc.Bacc(None, target_bir_lowering=False)
    xT = nc.declare_dram_parameter("xT", [DIM, N], BF16, isOutput=False)
    wq = nc.declare_dram_parameter("wq", [DIM, HL * D], BF16, isOutput=False)
    wkv = nc.declare_dram_parameter("wkv", [DIM, 2 * D], BF16, isOutput=False)
    wout = nc.declare_dram_parameter("wout", [HL * D, DIM], BF16, isOutput=False)
    cosq = nc.declare_dram_parameter("cosq", [D, N], BF16, isOutput=False)
    sinq = nc.declare_dram_parameter("sinq", [D, N], BF16, isOutput=False)
    cosk = nc.declare_dram_parameter("cosk", [D, N], BF16, isOutput=False)
    sink = nc.declare_dram_parameter("sink", [D, N], BF16, isOutput=False)
    # bf16 partials: the host sums 4 of them in f32; quantization error
    # (~0.4% rel) is well inside the 2e-2 gate and halves the y DMA traffic.
    y = nc.declare_dram_parameter("y", [N, DIM], BF16, isOutput=True)

    with ExitStack() as ctx:
        tc = ctx.enter_context(tile.TileContext(nc))
        consts = ctx.enter_context(tc.tile_pool(name="consts", bufs=1))
        xpool = ctx.enter_context(tc.tile_pool(name="xpool", bufs=2))
        proj = ctx.enter_context(tc.tile_pool(name="proj", bufs=2))
        sb = ctx.enter_context(tc.tile_pool(name="sb", bufs=3))
        outp = ctx.enter_context(tc.tile_pool(name="outp", bufs=2))
        # 8 PSUM banks: scores 2, proj/outproj work 3, psa 2, psd(+psdt view) 1
        ps_sc = ctx.enter_context(tc.tile_pool(name="ps_sc", bufs=2, space="PSUM"))
        ps_work = ctx.enter_context(tc.tile_pool(name="ps_work", bufs=3, space="PSUM"))
        ps_att = ctx.enter_context(tc.tile_pool(name="ps_att", bufs=2, space="PSUM"))
        ps_den = ctx.enter_context(tc.tile_pool(name="ps_den", bufs=1, space="PSUM"))

        ident = consts.tile([128, 128], BF16)
        make_identity(nc, ident)
        ones_col = consts.tile([128, 1], BF16)
        nc.vector.memset(ones_col, 1.0)
        zeros_h = consts.tile([128, HL], BF16)
        nc.vector.memset(zeros_h, 0.0)

        wq_sb = consts.tile([128, DC, HL * D], BF16)
        wkv_sb = consts.tile([128, DC, 2 * D], BF16)
        nc.sync.dma_start(
            wq_sb, wq.rearrange("(c p) m -> p c m", p=128))
        nc.sync.dma_start(
            wkv_sb, wkv.rearrange("(c p) m -> p c m", p=128))
        # bulk constants go on the ACT HWDGE queue so they don't delay the
        # x-tile stream on the SP queue
        wout_sb = consts.tile([128, HL, DIM], BF16)
        nc.scalar.dma_start(wout_sb, wout.rearrange("(c p) m -> p c m", p=128))
        cq_sb = consts.tile([128, N], BF16)
        sq_sb = consts.tile([128, N], BF16)
        ck_sb = consts.tile([128, N], BF16)
        sk_sb = consts.tile([128, N], BF16)
        nc.scalar.dma_start(cq_sb, cosq[:, :])
        nc.scalar.dma_start(sq_sb, sinq[:, :])
        nc.scalar.dma_start(ck_sb, cosk[:, :])
        nc.scalar.dma_start(sk_sb, sink[:, :])

        pend = []        # out-proj half-units carried across phases/reps
        npop = [0]       # pops since last parity realignment

        def pop_filler(on_act=False, reserve=0):
            if len(pend) > reserve:
                pend.pop(0)(on_act)
                npop[0] += 1

        for bi in range(reps):
            first = bi == 0
            qrot = proj.tile([128, HL, N], BF16, tag="qrot")
            krot = proj.tile([128, N], BF16, tag="krot")
            vnat = proj.tile([128, NKC, D], BF16, tag="vnat")
            attnT = proj.tile([128, HL, N], BF16, tag="attnT")

            def _attn_qtile(t, prev_fin, qrot=qrot, krot=krot,
                            vnat=vnat, attnT=attnT):
                # early tiles leave a reserve in the FIFO so the late tiles
                # (fed only by already-produced units) don't starve
                rsv = 40 if t < 12 else 0
                qsl = qrot[:, :, t * 128:(t + 1) * 128]
                psa = ps_att.tile([128, HL, 128], F32, tag="psa")

                def scores(j):
                    pss = ps_sc.tile([128, HL, 128], F32, tag="pss")
                    nc.tensor.matmul(pss, krot[:, j * 128:(j + 1) * 128], qsl,
                                     start=True, stop=True)
                    ex = sb.tile([128, HL, 128], BF16, name="ex",
                                 tag="exd" if j == t else "exp",
                                 bufs=None if j == t else 4)
                    nc.scalar.activation(ex, pss,
                                         mybir.ActivationFunctionType.Exp)
                    if j == t:
                        # diagonal chunk: keep where qc - kp >= 0
                        nc.gpsimd.affine_select(
                            out=ex, in_=ex,
                            compare_op=mybir.AluOpType.is_ge, fill=0.0,
                            base=0, pattern=[[0, HL], [1, 128]],
                            channel_multiplier=-1)
                    return ex

                def attnv(j, ex, first_a, last_a, psd, pop=True):
                    nc.tensor.matmul(psa, vnat[:, j, :], ex,
                                     start=first_a, stop=last_a)
                    for h in range(HL):
                        nc.tensor.matmul(psd[:, h:h + 1], ex[:, h, :], ones_col,
                                         start=False,
                                         stop=(last_a and h == HL - 1),
                                         skip_group_check=True)
                    if pop:
                        pop_filler(reserve=rsv)

                # pipeline: diag scores first, then c0..c_{t-1} with attnV
                # lagging one chunk; diag attnV last (its exp+mask latency
                # hides behind the whole tile).  prev tile's finalize chain
                # issues right after the first scores: its PE transposes wait
                # on the tile-end psd drain (DVE), which s(diag) covers.
                exd = scores(t)
                if prev_fin is not None:
                    prev_fin()
                # a tile-start filler keeps PE ahead of the 2-bank scores
                # ring warmup (the 3rd pss claim waits on exp(1st) done)
                pop_filler(reserve=rsv)
                # one bank holds BOTH den views: the accumulator psd in f32
                # cols 0:HL and the transposed row psdt in bytes 1024+ (a
                # bf16 bitcast view).  The ring claim orders tile t+1's
                # zero-matmul after tile t's reciprocal read, so the regions
                # never overlap live data.  This frees a whole PSUM bank,
                # giving pswork a 3rd buffer.
                psdf = ps_den.tile([128, 512], F32, tag="psd")
                psd = psdf[:, 0:HL]
                # zeroing matmul opens psd's zero region: the four per-head
                # accumulation chains share the bank without tripping each
                # other's pending-zero marks
                nc.tensor.matmul(psd, ident, zeros_h, start=True,
                                 stop=False, skip_group_check=True)
                prev = None
                na = 0
                for j in range(t):
                    ex = scores(j)
                    if prev is not None:
                        attnv(prev[0], prev[1], first_a=(prev[0] == 0),
                              last_a=False, psd=psd, pop=(na > 0))
                        na += 1
                    prev = (j, ex)
                if prev is not None:
                    attnv(prev[0], prev[1], first_a=(prev[0] == 0),
                          last_a=False, psd=psd, pop=(na > 0))
                    na += 1
                attnv(t, exd, first_a=(t == 0), last_a=True, psd=psd)
                # early half of finalize: drain psd while its bank cools
                sd = sb.tile([128, HL], BF16, tag="sd")
                nc.vector.tensor_copy(sd, psd)

                def finalize(t=t, psa=psa, sd=sd, psdf=psdf):
                    # flip den to row orientation: four [128,1]->[1,128] PE
                    # transposes land side by side in one psum row (lazy
                    # has_written zeroing keeps earlier columns intact)
                    psdt = psdf[0:1, 256:512].bitcast(BF16).rearrange(
                        "p (h q) -> p h q", h=HL)
                    for h in range(HL):
                        nc.tensor.transpose(psdt[0:1, h, :], sd[:, h:h + 1],
                                            ident)
                    den = sb.tile([1, HL, 128], F32, tag="den")
                    nc.vector.reciprocal(den, psdt)
                    bc = sb.tile([128, HL, 128], F32, tag="bc")
                    nc.gpsimd.partition_broadcast(bc, den)
                    nc.vector.tensor_mul(attnT[:, :, t * 128:(t + 1) * 128],
                                         psa, bc)
                return finalize

            def _outproj_units(t, attnT=attnT):
                # one 128-row m-chunk of y, as 8 half-units (2 head-chunks
                # each).  The two halves of a unit share one pswork claim, so
                # they must pop with no foreign pswork claim in between (see
                # parity realignment after the attention loop).
                units = []
                state = {}

                def half(nso, hi, on_act, m=t, state=state):
                    if "ysb" not in state:
                        state["ysb"] = outp.tile([128, DIM], BF16,
                                                 name="ysb", tag="ysb")
                    ysb = state["ysb"]
                    if hi == 0:
                        state[nso] = ps_work.tile([128, 512], F32,
                                                  name="psy", tag="pswork")
                    psy = state[nso] if hi == 0 else state.pop(nso)
                    for hc in (0, 1) if hi == 0 else (2, 3):
                        nc.tensor.matmul(
                            psy, attnT[:, hc, m * 128:(m + 1) * 128],
                            wout_sb[:, hc, nso * 512:(nso + 1) * 512],
                            start=(hc == 0), stop=(hc == HL - 1))
                    if hi == 1:
                        # psum drains can't ride Pool (no PSUM access on HW);
                        # all pops happen in the attention phase where ACT is
                        # exp-saturated, so drains ride DVE
                        nc.vector.tensor_copy(
                            ysb[:, nso * 512:(nso + 1) * 512], psy)
                        if nso == DIM // 512 - 1:
                            deng = nc.sync if m % 2 == 0 else nc.scalar
                            deng.dma_start(y[m * 128:(m + 1) * 128, :], ysb)

                for nso in range(DIM // 512):
                    for hi in (0, 1):
                        units.append(
                            lambda on_act=False, nso=nso, hi=hi: half(nso, hi, on_act))
                return units

            # ---- projections + rope, one n-slice at a time ----
            def _slice(ns):
                sl = slice(ns * SL, (ns + 1) * SL)
                xt = xpool.tile([128, DC, SL], BF16, tag="xt")
                h_dc = DC // 2
                xt_src = xT.rearrange("(c p) n -> p c n", p=128)[:, :, sl]
                nc.sync.dma_start(xt[:, :h_dc, :], xt_src[:, :h_dc, :])
                eng2 = nc.sync if (first and ns == 0) else nc.scalar
                eng2.dma_start(xt[:, h_dc:, :], xt_src[:, h_dc:, :])
                # v first, produced natural: stat = xT key-chunk, moving =
                # Wkv v-columns; 4 accumulation regions share one bank.  The
                # ACT drains land during the q matmuls.
                psvn = ps_work.tile([128, KPS, D], F32, tag="pswork")
                for kc in range(KPS):
                    for dc in range(DC):
                        nc.tensor.matmul(
                            psvn[:, kc, :], xt[:, dc, kc * 128:(kc + 1) * 128],
                            wkv_sb[:, dc, D:2 * D],
                            start=(dc == 0), stop=(dc == DC - 1),
                            skip_group_check=True)
                    nc.scalar.copy(vnat[:, ns * KPS + kc, :], psvn[:, kc, :])

                def qproj(hq):
                    psq = ps_work.tile([128, SL], F32, tag="pswork")
                    for dc in range(DC):
                        nc.tensor.matmul(
                            psq, wq_sb[:, dc, hq * D:(hq + 1) * D], xt[:, dc, :],
                            start=(dc == 0), stop=(dc == DC - 1))
                    _rope(nc, sb, psq, qrot[:, hq, sl],
                          cq_sb[:, sl], sq_sb[:, sl])

                for hq in range(HL):
                    qproj(hq)
                psk = ps_work.tile([128, SL], F32, tag="pswork")
                for dc in range(DC):
                    nc.tensor.matmul(
                        psk, wkv_sb[:, dc, 0:D], xt[:, dc, :],
                        start=(dc == 0), stop=(dc == DC - 1))
                _rope(nc, sb, psk, krot[:, sl],
                      ck_sb[:, sl], sk_sb[:, sl])

            # ---- attention tiles, software-pipelined ----
            # units(t) enter the FIFO after tile t and pop from tile t+1 on,
            # so finalize(t) has always issued before a unit reads attnT(t)
            fin = None
            npop[0] = 0
            for ns in range(NS):
                _slice(ns)
            for t in range(NQT):
                fin = _attn_qtile(t, fin)
                pend.extend(_outproj_units(t))
            # realign pops to unit parity so the projection-phase pop pairs
            # never split a unit across foreign pswork claims
            if npop[0] % 2 == 1:
                pop_filler()
            fin()
            if bi == reps - 1:
                while pend:
                    pop_filler()

    nc.finalize()
    return nc


def make_host_inputs(x, Wq, Wkv, Wout, HL):
    """Shard + precompute per-core input maps (host side)."""
    B, N, DIM = x.shape
    bf = ml_dtypes.bfloat16
    inv = 1.0 / (10000.0 ** (np.arange(0, D, 2, dtype=np.float64) / D))
    fr = np.arange(N, dtype=np.float64)[:, None] * inv[None, :]
    pos = np.concatenate([fr, fr], axis=-1)              # [N, D]
    cos_t = np.cos(pos).T.astype(np.float32)             # [D, N]
    sin_t = np.sin(pos).T.astype(np.float32)
    sign = np.ones((D, 1), np.float32)
    sign[:D // 2] = -1.0
    sin_r = sin_t * sign            # fold rotate_half's sign into the table
    shared = dict(
        wkv=Wkv.astype(bf),
        cosq=np.ascontiguousarray(cos_t * SCALE).astype(bf),
        sinq=np.ascontiguousarray(sin_r * SCALE).astype(bf),
        cosk=cos_t.astype(bf), sink=sin_r.astype(bf))
    xTs = [np.ascontiguousarray(x[b].T).astype(bf) for b in range(B)]
    in_maps = []
    for c in range(N_CORES):
        b, g = (c // HGROUPS) % B, c % HGROUPS
        lo, hi = g * HL * D, (g + 1) * HL * D
        in_maps.append(dict(
            shared, xT=xTs[b],
            wq=np.ascontiguousarray(Wq[:, lo:hi]).astype(bf),
            wout=np.ascontiguousarray(Wout[lo:hi, :]).astype(bf)))
    return in_maps


def kernel(x, Wq, Wkv, Wout):
    B, N, DIM = x.shape
    HL = HEADS // HGROUPS
    nc = build_nc(N, DIM, HL)
    in_maps = make_host_inputs(x, Wq, Wkv, Wout, HL)
    res = run_bass_kernel_spmd(nc, in_maps, core_ids=list(range(N_CORES)))
    y = np.zeros((B, N, DIM), np.float32)
    for c, r in enumerate(res.results):
        y[c // HGROUPS] += r["y"].astype(np.float32)
    return y
